# revision 15
# baseline (speedup 1.0000x reference)
"""Trainium2 Bass kernel for nn_GAT_67765993996807.

Per-segment multi-head attention (G=20 segments, H=5 heads, head dim 2) over
U=256 tokens, followed by output projection, LeakyReLU and mean-reduction to
a [B, G] output.  Data-parallel over the batch dim across 8 NeuronCores.

Math reformulation (verified vs the jax reference):
  h~[u, f]   = [x[b, 5g+n, u] (n<5), 1]                      (6 features)
  scores^T   = h~ P2_gh h~^T          P2_gh = (A~ B~^T)^T / sqrt(2)
  E = exp(scores^T)  (no max-subtraction needed: |scores| <~ 30)
  z[11h+d,u] = sum_v E[v,u] * (h~ @ WVO_gh)[v, d]   (d<10;  d=10 -> ones = den)
  out^T[u,d] = sum_h z[11h+d, u] / z[11h+10, u]  + bo
  final      = mean_{u,d} leaky_relu_0.3(out)
"""
import sys
import numpy as np

sys.path.insert(0, "/opt/trn_rl_repo")

from contextlib import ExitStack  # noqa: E402
import dataclasses  # noqa: E402
import concourse.bass as bass  # noqa: E402
import concourse.bacc as bacc  # noqa: E402
import concourse.tile as tile  # noqa: E402
from concourse import mybir  # noqa: E402
from concourse._compat import with_exitstack  # noqa: E402

B, S, U = 32, 100, 256
NG, G, D, H, K = 5, 20, 10, 5, 2
NCORES = 8
BL = B // NCORES          # 4 local batches per core
NBG = BL * G              # 80 (b,g) pairs per core
NEG = 0.3
F32 = mybir.dt.float32


# --------------------------------------------------------------------------
# host-side weight preparation (tiny: a few KB of numpy)
# --------------------------------------------------------------------------
def _host_prep(Wq, bq, Wk, bk, Wv, bv, Wo, bo):
    f32 = np.float32

    def eff(W):  # pairwise-duplicated columns fold: [G, D, H, K] -> [G, NG, H, K]
        return (W[:, 0::2] + W[:, 1::2]).astype(f32)

    # augmented (feature+bias) projections, heads-major: [G, H, 6, K]
    At = np.concatenate([eff(Wq).transpose(0, 2, 1, 3), bq[:, :, None, :]], axis=2)
    Bt = np.concatenate([eff(Wk).transpose(0, 2, 1, 3), bk[:, :, None, :]], axis=2)
    Ct = np.concatenate([eff(Wv).transpose(0, 2, 1, 3), bv[:, :, None, :]], axis=2)

    # P2[g,h] = (At Bt^T)^T / sqrt(2):  scores^T = h~ P2 h~^T
    P2 = np.einsum("ghfk,ghek->ghef", At, Bt) / np.sqrt(f32(K))  # [G,H,6,6]
    WVO = np.einsum("ghfk,ghkd->ghfd", Ct, Wo.astype(f32))       # [G,H,6,10]

    # stage-A stationary: p2a[g, f', h*6+e] = P2[g,h,e,f']
    p2a = np.ascontiguousarray(P2.transpose(0, 3, 1, 2).reshape(G, 6, H * 6))

    # stage-V moving operand, zero-padded head blocks of 55 cols:
    # block h occupies cols [55h, 55h+55); within it cols 11h..11h+10 are real:
    # 10 WVO columns + a ones-column (denominator).
    wvop = np.zeros((G, 6, H * 55), f32)
    for h in range(H):
        base = 55 * h + 11 * h
        wvop[:, :, base:base + 10] = WVO[:, h]
        wvop[:, 5, base + 10] = 1.0

    # bias tile: col g*11+d = bo[g,d] for d<10, 0 at d=10; then U ones
    bop = np.zeros((1, G * 11 + U), f32)
    bop[0, :G * 11].reshape(G, 11)[:, :10] = bo.astype(f32)
    bop[0, G * 11:] = 1.0

    ident = np.eye(128, dtype=f32)
    return p2a, wvop, bop, ident


# --------------------------------------------------------------------------
# device kernel body
# --------------------------------------------------------------------------
def _bcast_inner(ap: bass.AP, reps: int) -> bass.AP:
    """Append a step-0 (broadcast) innermost free dim."""
    return dataclasses.replace(ap, ap=list(ap.ap) + [[0, reps]])


@with_exitstack
def _body(ctx: ExitStack, tc: tile.TileContext, xs, p2a, wvop, bop, ident, out):
    nc = tc.nc
    AF = mybir.ActivationFunctionType
    OP = mybir.AluOpType

    singles = ctx.enter_context(tc.tile_pool(name="singles", bufs=1))
    sb = ctx.enter_context(tc.tile_pool(name="sb", bufs=3))
    espool = ctx.enter_context(tc.tile_pool(name="es", bufs=2))
    pssmall = ctx.enter_context(tc.tile_pool(name="pssmall", bufs=3, space="PSUM"))
    psscore = ctx.enter_context(tc.tile_pool(name="psscore", bufs=1, space="PSUM"))
    pszp = ctx.enter_context(tc.tile_pool(name="pszp", bufs=2, space="PSUM"))
    drp = ctx.enter_context(tc.tile_pool(name="drp", bufs=3, space="DRAM"))

    # ---------------- persistent SBUF state ----------------
    XA = []
    for b in range(BL):
        xa = singles.tile([6, G * U], F32, tag=f"xa{b}")
        onesrow = bop[0, G * 11:G * 11 + U]
        nc.sync.dma_start(
            out=xa[5:6, :].rearrange("p (g u) -> p g u", u=U),
            in_=dataclasses.replace(onesrow, ap=[[0, G]] + list(onesrow.ap)),
        )
        nc.sync.dma_start(
            out=xa[0:5, :].rearrange("f (g u) -> f g u", u=U),
            in_=xs[b].rearrange("(g f) u -> f g u", f=NG),
        )
        XA.append(xa)

    P2S = singles.tile([6, G * 30], F32, tag="p2s")
    nc.sync.dma_start(
        out=P2S[:].rearrange("f (g m) -> f g m", m=30),
        in_=p2a.rearrange("g f m -> f g m"),
    )
    WVOS = singles.tile([6, G * 275], F32, tag="wvos")
    nc.sync.dma_start(
        out=WVOS[:].rearrange("f (g m) -> f g m", m=275),
        in_=wvop.rearrange("g f m -> f g m"),
    )
    BOS = singles.tile([128, G * 11], F32, tag="bos")
    nc.sync.dma_start(
        out=BOS[:],
        in_=dataclasses.replace(bop[0, 0:G * 11],
                                ap=[[0, 128]] + list(bop[0, 0:G * 11].ap)),
    )
    IDT = singles.tile([128, 128], F32, tag="idt")
    nc.sync.dma_start(out=IDT[:], in_=ident[:, :])
    ONES = singles.tile([128, 1], F32, tag="ones")
    nc.vector.memset(ONES[:], 1.0 / float(D * U))
    FT = singles.tile([128, NBG], F32, tag="ft")

    # ---------------- main loop over (b, g) ----------------
    for bg in range(NBG):
        b, g = divmod(bg, G)
        xa = XA[b]

        # stage A: TH[(h,e), u] = sum_f' P2[g,h,e,f'] h~^T[f',u]
        aps = pssmall.tile([30, U], F32, tag="pss")
        nc.tensor.matmul(aps[:], lhsT=P2S[:, 30 * g:30 * g + 30],
                         rhs=xa[:, U * g:U * g + U], start=True, stop=True)
        araw = sb.tile([30, U], F32, tag="araw")
        nc.vector.tensor_copy(out=araw[:], in_=aps[:])
        # repack [30, U] -> [6, H*U] via a DRAM round-trip (partition remap)
        adr = drp.tile([30, U], F32, tag="adr")
        nc.sync.dma_start(out=adr[:], in_=araw[:])
        ths = sb.tile([6, H * U], F32, tag="ths")
        nc.sync.dma_start(
            out=ths[:].rearrange("f (h u) -> f h u", u=U),
            in_=adr[:].rearrange("(h f) u -> f h u", f=6),
        )

        zps = pszp.tile([55, U], F32, tag="psz")
        for c in range(2):
            xsl = xa[:, U * g + 128 * c: U * g + 128 * c + 128]
            # stage V: padded [VW | ones] blocks, [128, 275]
            vps = pssmall.tile([128, 275], F32, tag="pss")
            nc.tensor.matmul(vps[:], lhsT=xsl,
                             rhs=WVOS[:, 275 * g:275 * g + 275],
                             start=True, stop=True)
            vws = sb.tile([128, 275], F32, tag="vws")
            nc.vector.tensor_copy(out=vws[:], in_=vps[:])
            # stage S: scores^T chunks for all 5 heads -> one psum run [128, 1280]
            sps = psscore.tile([128, H * U], F32, tag="pssc")
            for h in range(H):
                nc.tensor.matmul(sps[:, U * h:U * h + U], lhsT=xsl,
                                 rhs=ths[:, U * h:U * h + U],
                                 start=True, stop=True)
            # stage E: exp of the whole 5-head chunk in one ACT op
            es = espool.tile([128, H * U], F32, tag="es")
            nc.scalar.activation(out=es[:], in_=sps[:], func=AF.Exp)
            # stage Z: z[55, U] accumulated over chunks and heads
            for h in range(H):
                nc.tensor.matmul(zps[:], lhsT=vws[:, 55 * h:55 * h + 55],
                                 rhs=es[:, U * h:U * h + U],
                                 start=(c == 0 and h == 0),
                                 stop=(c == 1 and h == H - 1))

        zs = sb.tile([55, U], F32, tag="zs")
        nc.vector.tensor_copy(out=zs[:], in_=zps[:])

        # transpose both u-halves -> one [u(128), 2*55] tile
        zt = sb.tile([128, 110], F32, tag="zt")
        for half in range(2):
            tps = pssmall.tile([128, 55], F32, tag="pss")
            nc.tensor.transpose(tps[:], zs[:, 128 * half:128 * half + 128],
                                IDT[0:55, 0:55])
            nc.vector.tensor_copy(out=zt[:, 55 * half:55 * half + 55], in_=tps[:])
        # r = 1 / den  (den at cols 55t + 11h + 10)
        r = sb.tile([128, 2 * H], F32, tag="r")
        den = zt[:].rearrange("p (t h e) -> p t h e", t=2, e=11)[:, :, :, 10]
        nc.vector.reciprocal(out=r[:], in_=den)
        # zn = zt * r (r broadcast over each head's 11 cols)
        zn = sb.tile([128, 110], F32, tag="zn")
        nc.vector.tensor_mul(out=zn[:], in0=zt[:], in1=_bcast_inner(r[:], 11))
        # w[t, dd] = sum_h zn[55t + 11h + dd]
        w = sb.tile([128, 22], F32, tag="w")
        nc.vector.tensor_reduce(
            out=w[:],
            in_=dataclasses.replace(zn[:], ap=[zn[:].ap[0], [55, 2], [1, 11], [11, 5]]),
            axis=mybir.AxisListType.X, op=OP.add)
        # + bo (broadcast over the two halves)
        w2 = sb.tile([128, 22], F32, tag="w2")
        bos_g = BOS[:, 11 * g:11 * g + 11]
        nc.vector.tensor_add(
            out=w2[:], in0=w[:],
            in1=dataclasses.replace(bos_g, ap=[bos_g.ap[0], [0, 2], [1, 11]]))
        # leaky relu on the 2*10 real cols + free-dim sum into FT[:, bg]
        lr = sb.tile([128, 20], F32, tag="lr")
        w2v = dataclasses.replace(w2[:], ap=[w2[:].ap[0], [11, 2], [1, 10]])
        nc.vector.scalar_tensor_tensor(
            out=lr[:], in0=w2v, scalar=NEG, in1=w2v,
            op0=OP.mult, op1=OP.max,
            accum_out=FT[:, bg:bg + 1])

    # ---------------- final reduction ----------------
    fps = pssmall.tile([1, NBG], F32, tag="pss")
    nc.tensor.matmul(fps[:], lhsT=ONES[:], rhs=FT[:], start=True, stop=True)
    fs = sb.tile([1, NBG], F32, tag="fs")
    nc.vector.tensor_copy(out=fs[:], in_=fps[:])
    nc.sync.dma_start(out=out[:, :], in_=fs[:])


# --------------------------------------------------------------------------
# build + run
# --------------------------------------------------------------------------
_CACHE = {}


def _build():
    if "nc" in _CACHE:
        return _CACHE["nc"], _CACHE["aps"]
    nc = bacc.Bacc("TRN2")
    xs = nc.dram_tensor("xs", [BL, S, U], F32, kind="ExternalInput")
    p2a = nc.dram_tensor("p2a", [G, 6, 30], F32, kind="ExternalInput")
    wvop = nc.dram_tensor("wvop", [G, 6, 275], F32, kind="ExternalInput")
    bop = nc.dram_tensor("bop", [1, G * 11 + U], F32, kind="ExternalInput")
    ident = nc.dram_tensor("ident", [128, 128], F32, kind="ExternalInput")
    out = nc.dram_tensor("out", [1, NBG], F32, kind="ExternalOutput")
    with tile.TileContext(nc) as tc:
        _body(tc, xs[:], p2a[:], wvop[:], bop[:], ident[:], out[:])
    nc.compile()
    _CACHE["nc"] = nc
    _CACHE["aps"] = (xs, p2a, wvop, bop, ident, out)
    return nc, _CACHE["aps"]


def kernel(x, Wq, bq, Wk, bk, Wv, bv, Wo, bo):
    x = np.ascontiguousarray(np.asarray(x, dtype=np.float32))
    p2a, wvop, bop, ident = _host_prep(
        *[np.asarray(t, dtype=np.float32) for t in (Wq, bq, Wk, bk, Wv, bv, Wo, bo)])

    nc, _ = _build()
    in_maps = []
    for core in range(NCORES):
        in_maps.append({
            "xs": np.ascontiguousarray(x[core * BL:(core + 1) * BL]),
            "p2a": p2a, "wvop": wvop, "bop": bop, "ident": ident,
        })

    from concourse.bass_utils import run_bass_kernel_spmd
    import os
    trace = bool(os.environ.get("BASS_GAT_TRACE"))
    res = run_bass_kernel_spmd(nc, in_maps, core_ids=list(range(NCORES)),
                               trace=trace)
    global LAST_RESULT
    LAST_RESULT = res
    out = np.concatenate([r["out"].reshape(BL, G) for r in res.results], axis=0)
    return out.astype(np.float32)


LAST_RESULT = None


if __name__ == "__main__":
    rng = np.random.default_rng(0)
    x = rng.normal(size=(B, S, U)).astype(np.float32)


# revision 38
# speedup vs baseline: 3.7633x; 3.7633x over previous
"""Trainium2 Bass kernel for nn_GAT_67765993996807.

Per-segment multi-head attention (G=20 segments, H=5 heads, head dim 2) over
U=256 tokens, followed by output projection, LeakyReLU and mean-reduction to
a [B, G] output.  Data-parallel over the batch dim across 8 NeuronCores.

Math reformulation (verified vs the jax reference):
  h~[u, f]   = [x[b, 5g+n, u] (n<5), 1]                      (6 features)
  scores^T   = h~ P2_gh h~^T          P2_gh = (A~ B~^T)^T / sqrt(2)
  E = exp(scores^T)  (no max-subtraction needed: |scores| <~ 30)
  z[11h+d,u] = sum_v E[v,u] * (h~ @ WVO_gh)[v, d]   (d<10;  d=10 -> ones = den)
  out^T[u,d] = sum_h z[11h+d, u] / z[11h+10, u]  + bo
  final      = mean_{u,d} leaky_relu_0.3(out)
"""
import sys
import numpy as np

sys.path.insert(0, "/opt/trn_rl_repo")

from contextlib import ExitStack  # noqa: E402
import dataclasses  # noqa: E402
import concourse.bass as bass  # noqa: E402
import concourse.bacc as bacc  # noqa: E402
import concourse.tile as tile  # noqa: E402
from concourse import mybir  # noqa: E402
from concourse._compat import with_exitstack  # noqa: E402

B, S, U = 32, 100, 256
NG, G, D, H, K = 5, 20, 10, 5, 2
NCORES = 8
BL = B // NCORES          # 4 local batches per core
NBG = BL * G              # 80 (b,g) pairs per core
NEG = 0.3
F32 = mybir.dt.float32


# --------------------------------------------------------------------------
# host-side weight preparation (tiny: a few KB of numpy)
# --------------------------------------------------------------------------
def _host_prep(Wq, bq, Wk, bk, Wv, bv, Wo, bo):
    f32 = np.float32

    def eff(W):  # pairwise-duplicated columns fold: [G, D, H, K] -> [G, NG, H, K]
        return (W[:, 0::2] + W[:, 1::2]).astype(f32)

    # augmented (feature+bias) projections, heads-major: [G, H, 6, K]
    At = np.concatenate([eff(Wq).transpose(0, 2, 1, 3), bq[:, :, None, :]], axis=2)
    Bt = np.concatenate([eff(Wk).transpose(0, 2, 1, 3), bk[:, :, None, :]], axis=2)
    Ct = np.concatenate([eff(Wv).transpose(0, 2, 1, 3), bv[:, :, None, :]], axis=2)

    # P2[g,h] = (At Bt^T)^T / sqrt(2):  scores^T = h~ P2 h~^T
    P2 = np.einsum("ghfk,ghek->ghef", At, Bt) / np.sqrt(f32(K))  # [G,H,6,6]
    WVO = np.einsum("ghfk,ghkd->ghfd", Ct, Wo.astype(f32))       # [G,H,6,10]

    # stage-A stationary, zero-padded to M=128 (fp32r needs col_grp=0xf):
    # p2a[g, f', h*6+e] = P2[g,h,e,f'] in cols 0..29, zeros elsewhere
    p2a = np.zeros((G, 6, 128), f32)
    p2a[:, :, :30] = P2.transpose(0, 3, 1, 2).reshape(G, 6, H * 6)

    # stage-V moving operand, zero-padded head blocks of 55 cols (+1 pad col
    # for fp32r even-innermost-count): block h occupies cols [55h, 55h+55);
    # within it cols 11h..11h+10 are real: 10 WVO columns + a ones-column.
    wvop = np.zeros((G, 6, H * 55 + 1), f32)
    for h in range(H):
        base = 55 * h + 11 * h
        wvop[:, :, base:base + 10] = WVO[:, h]
        wvop[:, 5, base + 10] = 1.0

    # bias tile: col g*11+d = bo[g,d] for d<10, 0 at d=10; then U ones
    bop = np.zeros((1, G * 11 + U), f32)
    bop[0, :G * 11].reshape(G, 11)[:, :10] = bo.astype(f32)
    bop[0, G * 11:] = 1.0

    ident = np.eye(128, dtype=f32)
    return p2a, wvop, bop, ident


# --------------------------------------------------------------------------
# device kernel body
# --------------------------------------------------------------------------
def _bcast_inner(ap: bass.AP, reps: int) -> bass.AP:
    """Append a step-0 (broadcast) innermost free dim."""
    return dataclasses.replace(ap, ap=list(ap.ap) + [[0, reps]])


F32R = mybir.dt.float32r
BF16 = mybir.dt.bfloat16


@with_exitstack
def _body(ctx: ExitStack, tc: tile.TileContext, xs, p2a, wvop, bop, ident, out,
          reps: int = 1):
    nc = tc.nc
    AF = mybir.ActivationFunctionType
    OP = mybir.AluOpType

    singles = ctx.enter_context(tc.tile_pool(name="singles", bufs=1))
    sb = ctx.enter_context(tc.tile_pool(name="sb", bufs=3))
    sbxa = ctx.enter_context(tc.tile_pool(name="sbxa", bufs=2))
    sbth = ctx.enter_context(tc.tile_pool(name="sbth", bufs=2))
    espool = ctx.enter_context(tc.tile_pool(name="es", bufs=4))
    pssmall = ctx.enter_context(tc.tile_pool(name="pssmall", bufs=2, space="PSUM"))
    psscore = ctx.enter_context(tc.tile_pool(name="psscore", bufs=2, space="PSUM"))
    drp = ctx.enter_context(tc.tile_pool(name="drp", bufs=2, space="DRAM"))

    P2S = singles.tile([6, G * 128], F32R, tag="p2s")
    nc.sync.dma_start(
        out=P2S[:].rearrange("f (g m) -> f g m", m=128),
        in_=p2a.rearrange("g f m -> f g m"),
    )
    WVOS = singles.tile([6, G * 276], F32R, tag="wvos")
    nc.sync.dma_start(
        out=WVOS[:].rearrange("f (g m) -> f g m", m=276),
        in_=wvop.rearrange("g f m -> f g m"),
    )
    BOS = singles.tile([128, G * 11], F32, tag="bos")
    nc.sync.dma_start(
        out=BOS[:],
        in_=dataclasses.replace(bop[0, 0:G * 11],
                                ap=[[0, 128]] + list(bop[0, 0:G * 11].ap)),
    )
    IDT = singles.tile([128, 128], F32, tag="idt")
    nc.sync.dma_start(out=IDT[:], in_=ident[:, :])
    ONES = singles.tile([128, 1], F32, tag="ones")
    nc.vector.memset(ONES[:], 1.0 / float(D * U))
    FT = singles.tile([128, NBG], F32, tag="ft")

    # ---------------- pipelined main loop ----------------
    # batches of NB (b,g) pairs; stage A for batch k+1 is prepared while
    # batch k computes; the Z/tail stage runs one bg behind the S/E stage
    # so the PE never stalls on the exp latency.
    NB = 5
    nbatch = NBG // NB
    xa_tiles: dict = {}

    def load_xa(b):
        xa = sbxa.tile([6, G * U], F32R, tag="xa")
        onesrow = bop[0, G * 11:G * 11 + U].bitcast(F32R)
        nc.sync.dma_start(
            out=xa[5:6, :].rearrange("p (g u) -> p g u", u=U),
            in_=dataclasses.replace(onesrow, ap=[[0, G]] + list(onesrow.ap)),
        )
        nc.gpsimd.dma_start(
            out=xa[0:5, :].rearrange("f (g u) -> f g u", u=U),
            in_=xs[b].rearrange("(g f) u -> f g u", f=NG),
        )
        xa_tiles[b] = xa

    def prepare(k):
        """Stage A for batch k: TH for its NB bgs, repacked to [6, (j h u)]."""
        bg0 = k * NB
        b = bg0 // G
        if b not in xa_tiles:
            load_xa(b)
        xa = xa_tiles[b]
        araw = sb.tile([30, NB * U], F32R, tag="araw")
        for j in range(NB):
            g = (bg0 + j) % G
            aps = pssmall.tile([128, U], F32, tag="pss")
            nc.tensor.matmul(aps[:], lhsT=P2S[:, 128 * g:128 * g + 128],
                             rhs=xa[:, U * g:U * g + U],
                             start=True, stop=True)
            nc.vector.tensor_copy(out=araw[:, U * j:U * j + U], in_=aps[0:30, :])
        adr = drp.tile([30, NB * U], F32R, tag="adr")
        nc.sync.dma_start(out=adr[:], in_=araw[:])
        # repack to [6, (h j u)]: dest is contiguous per (h,f) -> few descriptors
        ths = sbth.tile([6, NB * H * U], F32R, tag="ths")
        nc.sync.dma_start(
            out=ths[:].rearrange("f (h ju) -> f h ju", ju=NB * U),
            in_=adr[:].rearrange("(h f) ju -> f h ju", f=6),
        )
        return ths

    def emit_sve(bg, j, ths):
        """V, S, exp for one bg; returns handles needed by the Z stage."""
        b, g = divmod(bg, G)
        xa = xa_tiles[b]
        vws_c, es_c = [], []
        for c in range(2):
            xsl = xa[:, U * g + 128 * c: U * g + 128 * c + 128]
            vps = pssmall.tile([128, 276], F32, tag="pss")
            nc.tensor.matmul(vps[:], lhsT=xsl,
                             rhs=WVOS[:, 276 * g:276 * g + 276],
                             start=True, stop=True)
            vws = espool.tile([128, 275], BF16, tag="vws")
            nc.vector.tensor_copy(out=vws[:], in_=vps[:, 0:275])
            sps = psscore.tile([128, H * U], F32, tag="pssc")
            for h in range(H):
                src = (h * NB + j) * U
                nc.tensor.matmul(
                    sps[:, U * h:U * (h + 1)], lhsT=xsl,
                    rhs=ths[:, src:src + U],
                    start=True, stop=True)
            es = espool.tile([128, H * U], BF16, tag="es")
            nc.scalar.activation(out=es[:], in_=sps[:], func=AF.Exp)
            vws_c.append(vws)
            es_c.append(es)
        return (bg, vws_c, es_c)

    def emit_ztail(pend):
        """Z accumulation, transpose and normalization tail for one bg."""
        bg, vws_c, es_c = pend
        g = bg % G
        zps = pssmall.tile([55, U], F32, tag="pss")
        for c in range(2):
            for h in range(H):
                nc.tensor.matmul(zps[:], lhsT=vws_c[c][:, 55 * h:55 * h + 55],
                                 rhs=es_c[c][:, U * h:U * h + U],
                                 start=(c == 0 and h == 0),
                                 stop=(c == 1 and h == H - 1))
        zs = sb.tile([55, U], F32, tag="zs")
        nc.vector.tensor_copy(out=zs[:], in_=zps[:])
        tps = pssmall.tile([128, 110], F32, tag="pss")
        for half in range(2):
            nc.tensor.transpose(tps[:, 55 * half:55 * half + 55],
                                zs[:, 128 * half:128 * half + 128],
                                IDT[0:55, 0:55])
        # r = 1 / den  (den at cols 55t + 11h + 10)
        r = sb.tile([128, 2 * H], F32, tag="r")
        den = tps[:].rearrange("p (t h e) -> p t h e", t=2, e=11)[:, :, :, 10]
        nc.vector.reciprocal(out=r[:], in_=den)
        zn = sb.tile([128, 110], F32, tag="zn")
        nc.vector.tensor_mul(out=zn[:], in0=tps[:], in1=_bcast_inner(r[:], 11))
        w = sb.tile([128, 22], F32, tag="w")
        nc.vector.tensor_reduce(
            out=w[:],
            in_=dataclasses.replace(zn[:], ap=[zn[:].ap[0], [55, 2], [1, 11], [11, 5]]),
            axis=mybir.AxisListType.X, op=OP.add)
        w2 = sb.tile([128, 22], F32, tag="w2")
        bos_g = BOS[:, 11 * g:11 * g + 11]
        nc.vector.tensor_add(
            out=w2[:], in0=w[:],
            in1=dataclasses.replace(bos_g, ap=[bos_g.ap[0], [0, 2], [1, 11]]))
        lr = sb.tile([128, 20], F32, tag="lr")
        w2v = dataclasses.replace(w2[:], ap=[w2[:].ap[0], [11, 2], [1, 10]])
        nc.vector.scalar_tensor_tensor(
            out=lr[:], in0=w2v, scalar=NEG, in1=w2v,
            op0=OP.mult, op1=OP.max,
            accum_out=FT[:, bg:bg + 1])

    for _rep in range(reps):
        xa_tiles.clear()
        ths_cur = prepare(0)
        pend = None
        for k in range(nbatch):
            ths_next = prepare(k + 1) if k + 1 < nbatch else None
            for j in range(NB):
                bg = k * NB + j
                sve = emit_sve(bg, j, ths_cur)
                if pend is not None:
                    emit_ztail(pend)
                pend = sve
            ths_cur = ths_next
        emit_ztail(pend)

    # ---------------- final reduction ----------------
    fps = pssmall.tile([1, NBG], F32, tag="pss")
    nc.tensor.matmul(fps[:], lhsT=ONES[:], rhs=FT[:], start=True, stop=True)
    fs = sb.tile([1, NBG], F32, tag="fs")
    nc.vector.tensor_copy(out=fs[:], in_=fps[:])
    nc.sync.dma_start(out=out[:, :], in_=fs[:])


# --------------------------------------------------------------------------
# build + run
# --------------------------------------------------------------------------
_CACHE = {}


def _build(reps: int = 1):
    key = f"nc{reps}"
    if key in _CACHE:
        return _CACHE[key], _CACHE["aps" + key]
    nc = bacc.Bacc("TRN2")
    xs = nc.dram_tensor("xs", [BL, S, U], F32R, kind="ExternalInput")
    p2a = nc.dram_tensor("p2a", [G, 6, 128], F32R, kind="ExternalInput")
    wvop = nc.dram_tensor("wvop", [G, 6, 276], F32R, kind="ExternalInput")
    bop = nc.dram_tensor("bop", [1, G * 11 + U], F32, kind="ExternalInput")
    ident = nc.dram_tensor("ident", [128, 128], F32, kind="ExternalInput")
    out = nc.dram_tensor("out", [1, NBG], F32, kind="ExternalOutput")
    with tile.TileContext(nc) as tc:
        _body(tc, xs[:], p2a[:], wvop[:], bop[:], ident[:], out[:], reps=reps)
    nc.compile()
    _CACHE[key] = nc
    _CACHE["aps" + key] = (xs, p2a, wvop, bop, ident, out)
    return nc, _CACHE["aps" + key]


def kernel(x, Wq, bq, Wk, bk, Wv, bv, Wo, bo):
    x = np.ascontiguousarray(np.asarray(x, dtype=np.float32))
    p2a, wvop, bop, ident = _host_prep(
        *[np.asarray(t, dtype=np.float32) for t in (Wq, bq, Wk, bk, Wv, bv, Wo, bo)])

    nc, _ = _build()
    in_maps = []
    for core in range(NCORES):
        in_maps.append({
            "xs": np.ascontiguousarray(x[core * BL:(core + 1) * BL]),
            "p2a": p2a, "wvop": wvop, "bop": bop, "ident": ident,
        })

    from concourse.bass_utils import run_bass_kernel_spmd
    import os
    trace = bool(os.environ.get("BASS_GAT_TRACE"))
    res = run_bass_kernel_spmd(nc, in_maps, core_ids=list(range(NCORES)),
                               trace=trace)
    global LAST_RESULT
    LAST_RESULT = res
    out = np.concatenate([r["out"].reshape(BL, G) for r in res.results], axis=0)
    return out.astype(np.float32)


LAST_RESULT = None


if __name__ == "__main__":
    rng = np.random.default_rng(0)
    x = rng.normal(size=(B, S, U)).astype(np.float32)
